# revision 5
# baseline (speedup 1.0000x reference)
"""Trainium2 Bass kernel for nn_AttnBlock (B=4, C=256, T=4096) on 8 NeuronCores.

v2: fp8 DoubleRow attention + mask compaction.

Sharding: core = (batch b = core//2, query-half = core%2). The host compacts
each batch's time axis to its kept (mask=1) positions only — masked positions
are dead in the reference output (final *m) and as keys (softmax weight 0) —
then rolls the compacted axis so this core's query half sits at columns 0..Q.
Keys = all kept positions (padded to a multiple of 256), queries = this
core's half (padded to a multiple of 128).

Math foldings (exact):
  - gamma/beta/mean-subtraction fold into the conv weights (centered Wc).
  - The LayerNorm scale rstd commutes through every conv; the host folds it
    into the input once: x' = x * rstd. k/q/v then come straight from convs.
  - All biases are zero (asserted); Wo@bv+bo added on host.
  - softmax shift: e = exp(s/16 - SHIFT) fits fp8e4m3; shift cancels.
  - pad keys: x' columns are 0 so v-pad = 0 (kills AV) and the denominator
    ones-pattern has 0 rows at pads (kills the denom) — no -1e8 bias needed.

Dataflow per core (K_pad keys = NP pairs of 128-chunks, Q_pad queries in
tiles of <=512):
  convs (bf16, psum fp32): k8/q8 [c'(2x128), t] fp8; vt8 [s, chunk, c'] fp8
  per query tile jt, per key pair p:
    scores  = DoubleRow fp8: k8 pair -> sc [128, 2, qw] psum (2 banks)
    e       = one ACT exp per pair [128, 2, qw] -> fp8 (scalar bias/scale)
    AV m0   = DoubleRow fp8 into hpre bank;  m1 replayed after the loop
              (e tiles persist) to stay within 8 psum banks
    denom   = DoubleRow fp8 with ones-pattern weights -> dnm [16, qw] bank
  epilogue: hpre -> bf16, out^T = hpre^T @ Wo per 128-query chunk with
  per-partition 1/denom scale, bf16 DMA out.

ACT (exp) is the bottleneck engine (~59us); PE ~48us; DVE does all psum
drains (~39us); Pool cannot access PSUM.
"""
import sys

if "/opt/trn_rl_repo" not in sys.path:
    sys.path.insert(0, "/opt/trn_rl_repo")

import numpy as np
import ml_dtypes

import concourse.tile as tile
from concourse import bacc, mybir
from concourse.bass_utils import run_bass_kernel_spmd

B, C, T = 4, 256, 4096
N_CORES = 8
EPS = 1e-5
SCALE = float(C) ** -0.5
SHIFT = 4.0
BF16 = mybir.dt.bfloat16
F8 = mybir.dt.float8e4
F32 = mybir.dt.float32
NP_BF16 = ml_dtypes.bfloat16
NP_F8 = (ml_dtypes.float8_e4m3fn if hasattr(ml_dtypes, "float8_e4m3fn")
         else ml_dtypes.float8_e4m3)
DR = mybir.MatmulPerfMode.DoubleRow
AF = mybir.ActivationFunctionType


def _q_tiles(q_pad):
    """Query tile widths (each <=512, multiple of 128)."""
    tiles = [512] * (q_pad // 512)
    if q_pad % 512:
        tiles.append(q_pad % 512)
    return tiles


def build_kernel(k_pad, q_pad):
    npair = k_pad // 256
    nc = bacc.Bacc("TRN2", target_bir_lowering=False, debug=False,
                   num_devices=N_CORES)
    d_x2 = nc.dram_tensor("x2", [128, 2, k_pad], BF16, kind="ExternalInput").ap()
    d_x8 = nc.dram_tensor("x8", [128, 2, k_pad], F8, kind="ExternalInput").ap()
    d_u8 = nc.dram_tensor("u8", [128, 2, q_pad], F8, kind="ExternalInput").ap()
    d_w = nc.dram_tensor("wcat", [128, 2, 2, 256], BF16,
                         kind="ExternalInput").ap()
    d_w1 = nc.dram_tensor("w1", [128, npair, 2, 16], F8,
                          kind="ExternalInput").ap()
    d_out = nc.dram_tensor("out", [q_pad, C], BF16, kind="ExternalOutput").ap()
    with tile.TileContext(nc) as tc:
        _body(tc, d_x2, d_x8, d_u8, d_w, d_w1, d_out, k_pad, q_pad)
    nc.compile()
    return nc


def _body(tc, d_x2, d_x8, d_u8, d_w, d_w1, d_out, k_pad, q_pad):
    nc = tc.nc
    from contextlib import ExitStack

    npair = k_pad // 256
    qtiles = _q_tiles(q_pad)
    njt = len(qtiles)

    with ExitStack() as ctx:
        consts = ctx.enter_context(tc.tile_pool(name="consts", bufs=1))
        big = ctx.enter_context(tc.tile_pool(name="big", bufs=1))

        # ---- input DMAs (few large transfers) ----
        wcat = consts.tile([128, 2, 2, 256], BF16, tag="wcat")
        nc.gpsimd.dma_start(wcat[:], d_w[:])
        w1 = consts.tile([128, npair, 2, 16], F8, tag="w1")
        nc.gpsimd.dma_start(w1[:], d_w1[:])
        x8 = consts.tile([128, 2, k_pad], F8, tag="x8")
        u8 = consts.tile([128, 2, q_pad], F8, tag="u8")
        x2 = consts.tile([128, 2, k_pad], BF16, tag="x2")
        # u8 (host-computed G@x') + first x8 piece gate the first score
        # pair; x2 only feeds the v-convs
        nc.sync.dma_start(u8[:, :, 0:qtiles[0]], d_u8[:, :, 0:qtiles[0]])
        nc.sync.dma_start(x8[:, :, 0:512], d_x8[:, :, 0:512])
        nc.sync.dma_start(x2[:, :, 0:1024], d_x2[:, :, 0:1024])
        nc.sync.dma_start(x8[:, :, 512:k_pad], d_x8[:, :, 512:k_pad])
        if q_pad > qtiles[0]:
            nc.sync.dma_start(u8[:, :, qtiles[0]:q_pad],
                              d_u8[:, :, qtiles[0]:q_pad])
        bnd = [1024, 2048, 3072, k_pad]
        for lo, hi in zip(bnd[:-1], bnd[1:]):
            nc.sync.dma_start(x2[:, :, lo:hi], d_x2[:, :, lo:hi])

        wv, wo = (wcat[:, i] for i in range(2))
        biascol = consts.tile([128, 1], F32, tag="biascol")
        nc.vector.memset(biascol[:], -SHIFT)
        ones11 = consts.tile([1, 1], F32, tag="ones11")
        nc.vector.memset(ones11[:], 1.0)

        # persistent activations
        vt8 = big.tile([128, k_pad // 128, 256], F8, tag="vt8")

        # PSUM: sc 2x2 + hpre(m0) 1 + h1 1 + dnm 1 + bankp 1 = 8 banks
        scp = ctx.enter_context(tc.tile_pool(name="scp", bufs=2, space="PSUM"))
        hpp = ctx.enter_context(tc.tile_pool(name="hpp", bufs=1, space="PSUM"))
        hp1 = ctx.enter_context(tc.tile_pool(name="hp1", bufs=1, space="PSUM"))
        dnp = ctx.enter_context(tc.tile_pool(name="dnp", bufs=1, space="PSUM"))
        bankp = ctx.enter_context(tc.tile_pool(name="bankp", bufs=1,
                                               space="PSUM"))
        # SBUF pools
        e_pool = ctx.enter_context(tc.tile_pool(name="e_pool",
                                                bufs=npair + 4))
        hsb = ctx.enter_context(tc.tile_pool(name="hsb", bufs=2))
        s3t = ctx.enter_context(tc.tile_pool(name="s3t", bufs=3))
        s3o = ctx.enter_context(tc.tile_pool(name="s3o", bufs=2))

        # conv psum scratch alternates between bankp and hp1 (hp1 only
        # becomes the m1-accumulator after all convs are done)
        conv_pools = [bankp, hp1]
        conv_pi = [0]
        drain_alt = [0]

        def conv_drain(dst_ap, src_ap):
            nc.vector.tensor_copy(dst_ap, src_ap)

        def conv_psum():
            pool = conv_pools[conv_pi[0] % 2]
            conv_pi[0] += 1
            return pool.tile([128, 512], F32, tag="bank" if pool is bankp
                             else "h1", name="cp")

        # ---------------- conv units ------------------------------------
        def conv_kq(dst, w2, lo, hi):
            # dst[c'-half m, lo:hi] for m in 0,1; contraction over 2x128 c_in
            for mm in range(2):
                pp = conv_psum()
                p_ap = pp[:, 0:hi - lo]
                ms = slice(128 * mm, 128 * (mm + 1))
                nc.tensor.matmul(p_ap, w2[:, 0, ms], x2[:, 0, lo:hi],
                                 start=True, stop=False, skip_group_check=True)
                nc.tensor.matmul(p_ap, w2[:, 1, ms], x2[:, 1, lo:hi],
                                 start=False, stop=True, skip_group_check=True)
                conv_drain(dst[:, mm, lo:hi], p_ap)

        def conv_v(p):
            # v^T chunks 2p, 2p+1: [s, c] layout, one drain per pair
            vp = conv_psum()
            vv = vp.rearrange("p (two c) -> p two c", two=2)
            for cj in range(2):
                j = 2 * p + cj
                sl = slice(128 * j, 128 * (j + 1))
                nc.tensor.matmul(vv[:, cj], x2[:, 0, sl], wv[:, 0],
                                 start=True, stop=False, skip_group_check=True)
                nc.tensor.matmul(vv[:, cj], x2[:, 1, sl], wv[:, 1],
                                 start=False, stop=True, skip_group_check=True)
            conv_drain(vt8[:, 2 * p:2 * p + 2, :], vv[:])

        # conv unit list: interleaved into jt0's pair loop
        conv_units = []
        kbnd = list(range(0, k_pad, 512)) + [k_pad]
        kspans = list(zip(kbnd[:-1], kbnd[1:]))
        qbnd = list(range(0, q_pad, 512)) + [q_pad]
        qspans = list(zip(qbnd[:-1], qbnd[1:]))
        for p in range(npair):
            conv_units.append(lambda p=p: conv_v(p))

        # ---------------- attention building blocks ---------------------
        state = {}

        def open_jt(jt):
            state[jt] = {"e": {},
                         "h0": hpp.tile([128, 512], F32, tag="h", name="h0"),
                         "dnm": dnp.tile([16, 512], F32, tag="dnm",
                                         name="dnm"),
                         "hsb": hsb.tile([128, 2, 512], BF16, tag="hsb",
                                         name="hsb")}

        def sc_pair(jt, p):
            qlo = sum(qtiles[:jt])
            qw = qtiles[jt]
            sc = scp.tile([128, 2, 512], F32, tag="sc", name="sc")
            for i in range(2):
                ch = 2 * p + i
                nc.tensor.matmul(sc[:, i, 0:qw],
                                 x8[:, :, 128 * ch:128 * (ch + 1)],
                                 u8[:, :, qlo:qlo + qw],
                                 start=True, stop=True, perf_mode=DR,
                                 skip_group_check=True)
            e = e_pool.tile([128, 2, 512], F8, tag="e", name="e")
            nc.scalar.activation(e[:, :, 0:qw], sc[:, :, 0:qw], AF.Exp,
                                 bias=biascol[:], scale=SCALE)
            state[jt]["e"][p] = e

        def av_m0(jt, p):
            st = state[jt]
            qw = qtiles[jt]
            nc.tensor.matmul(st["h0"][:, 0:qw], vt8[:, 2 * p:2 * p + 2, 0:128],
                             st["e"][p][:, :, 0:qw], start=(p == 0),
                             stop=(p == npair - 1), perf_mode=DR,
                             skip_group_check=True)

        def dnm_p(jt, p):
            st = state[jt]
            qw = qtiles[jt]
            nc.tensor.matmul(st["dnm"][:, 0:qw], w1[:, p],
                             st["e"][p][:, :, 0:qw], start=(p == 0),
                             stop=(p == npair - 1), perf_mode=DR,
                             skip_group_check=True)

        def av_m1_open(jt):
            state[jt]["h1"] = hp1.tile([128, 512], F32, tag="h1", name="h1")

        def av_m1_p(jt, p, pop=True):
            st = state[jt]
            qw = qtiles[jt]
            e = st["e"].pop(p) if pop else st["e"][p]
            nc.tensor.matmul(st["h1"][:, 0:qw], vt8[:, 2 * p:2 * p + 2,
                                                    128:256],
                             e[:, :, 0:qw], start=(p == 0),
                             stop=(p == npair - 1), perf_mode=DR,
                             skip_group_check=True)

        def drain_h0(jt):
            st = state[jt]
            qw = qtiles[jt]
            nc.vector.tensor_copy(st["hsb"][:, 0, 0:qw], st["h0"][:, 0:qw])

        def drain_h1(jt):
            st = state[jt]
            qw = qtiles[jt]
            nc.vector.tensor_copy(st["hsb"][:, 1, 0:qw], st["h1"][:, 0:qw])

        def drain_drow(jt):
            st = state[jt]
            qw = qtiles[jt]
            drow = s3t.tile([1, 512], F32, tag="drow", name="drow")
            nc.vector.tensor_copy(drow[:, 0:qw], st["dnm"][0:1, 0:qw])
            st["drow"] = drow

        def rinv_jt(jt):
            st = state[jt]
            qw = qtiles[jt]
            nch = qw // 128
            dcol = bankp.tile([128, 512], F32, tag="bank", name="dcol")
            for c4 in range(nch):
                nc.tensor.matmul(dcol[:, c4:c4 + 1],
                                 st["drow"][0:1, 128 * c4:128 * (c4 + 1)],
                                 ones11[:], start=True, stop=True,
                                 skip_group_check=True)
            rinv = s3t.tile([128, 4], F32, tag="rinv", name="rinv")
            nc.vector.reciprocal(rinv[:, 0:nch], dcol[:, 0:nch])
            st["rinv"] = rinv

        def epilogue_open(jt):
            state[jt]["o_sb"] = s3o.tile([128, 4, 256], BF16, tag="o_sb",
                                         name="o_sb")

        def epilogue_chunk(jt, c4):
            st = state[jt]
            cs = slice(128 * c4, 128 * (c4 + 1))
            ot = bankp.tile([128, 512], F32, tag="bank", name="ot")
            nc.tensor.matmul(ot[:, 0:256], st["hsb"][:, 0, cs], wo[:, 0],
                             start=True, stop=False, skip_group_check=True)
            nc.tensor.matmul(ot[:, 0:256], st["hsb"][:, 1, cs], wo[:, 1],
                             start=False, stop=True, skip_group_check=True)
            nc.vector.tensor_scalar_mul(st["o_sb"][:, c4], ot[:, 0:256],
                                        st["rinv"][:, c4:c4 + 1])

        def epilogue_dma(jt):
            st = state.pop(jt)
            qlo = sum(qtiles[:jt])
            qw = qtiles[jt]
            nch = qw // 128
            dview = d_out[qlo:qlo + qw, :].rearrange("(c p) o -> p c o", p=128)
            nc.sync.dma_start(dview, st["o_sb"][:, 0:nch])

        # ---------------- emission --------------------------------------
        # jt0: conv units interleaved; av-m1 replayed at the jt0/jt1
        # boundary (hp1 doubles as conv scratch during jt0).
        # steady jts: av m0/m1/dnm all in-loop, lagged behind the scores so
        # prev-jt bank drains (emitted at jt start) land first.
        ci = 0

        def emit_convs(n):
            nonlocal ci
            for _ in range(n):
                if ci < len(conv_units):
                    conv_units[ci]()
                    ci += 1

        open_jt(0)
        sc_pair(0, 0)
        sc_pair(0, 1)
        emit_convs(1)               # v0
        for s in range(npair + 2):
            emit_convs(2)
            if s + 2 < npair:
                sc_pair(0, s + 2)
            if s >= 2:
                av_m0(0, s - 2)
                dnm_p(0, s - 2)
        emit_convs(len(conv_units))  # stragglers

        for jt in range(1, njt):
            open_jt(jt)
            prev = jt - 1
            sc_pair(jt, 0)
            sc_pair(jt, 1)
            if prev == 0:
                av_m1_open(0)       # replay jt0's m1 (e tiles persist)
                for p in range(npair):
                    av_m1_p(0, p)
            nch_prev = qtiles[prev] // 128
            for s in range(npair + 3):
                if s + 2 < npair:
                    sc_pair(jt, s + 2)
                if 2 <= s < npair + 2:
                    av_m0(jt, s - 2)
                    dnm_p(jt, s - 2)
                if s == 3:
                    av_m1_open(jt)
                if 3 <= s < npair + 3:
                    av_m1_p(jt, s - 3)
                if s == 0:
                    drain_h0(prev)
                elif s == 1:
                    drain_h1(prev)
                    drain_drow(prev)
                elif s == 2:
                    rinv_jt(prev)
                    epilogue_open(prev)
                elif 3 <= s < 3 + nch_prev:
                    epilogue_chunk(prev, s - 3)
                elif s == 3 + nch_prev:
                    epilogue_dma(prev)

        # ---- tail: last jt epilogue on the critical path ----
        last = njt - 1
        st = state[last]
        qw = qtiles[last]
        nch = qw // 128
        qlo = sum(qtiles[:last])
        drain_drow(last)            # gates the denom->rinv->scale chain
        nc.scalar.copy(st["hsb"][:, 1, 0:qw], st["h1"][:, 0:qw])   # ACT
        drain_h0(last)                                             # DVE
        rinv_jt(last)
        epilogue_open(last)
        dview = d_out[qlo:qlo + qw, :].rearrange("(c p) o -> p c o", p=128)
        for c4 in range(nch):
            cs = slice(128 * c4, 128 * (c4 + 1))
            pool = [bankp, hpp][c4 % 2]
            ot = pool.tile([128, 512], F32, tag="bank" if pool is bankp
                           else "h", name="ot")
            nc.tensor.matmul(ot[:, 0:256], st["hsb"][:, 0, cs], wo[:, 0],
                             start=True, stop=False, skip_group_check=True)
            nc.tensor.matmul(ot[:, 0:256], st["hsb"][:, 1, cs], wo[:, 1],
                             start=False, stop=True, skip_group_check=True)
            if c4 % 2 == 0:
                nc.vector.tensor_scalar_mul(st["o_sb"][:, c4], ot[:, 0:256],
                                            st["rinv"][:, c4:c4 + 1])
            else:
                nc.scalar.activation(st["o_sb"][:, c4], ot[:, 0:256],
                                     AF.Copy, bias=0.0,
                                     scale=st["rinv"][:, c4:c4 + 1])
            eng = [nc.sync, nc.gpsimd, nc.scalar][c4 % 3]
            eng.dma_start(dview[:, c4:c4 + 1], st["o_sb"][:, c4:c4 + 1])
        state.pop(last)


_NC_CACHE = {}


def _get_nc(k_pad, q_pad):
    key = (k_pad, q_pad)
    if key not in _NC_CACHE:
        _NC_CACHE[key] = build_kernel(k_pad, q_pad)
    return _NC_CACHE[key]


def _chunk_pf(a):
    """[256, n] -> [128, 2, n] partition-first bf16."""
    n = a.shape[1]
    return np.ascontiguousarray(
        a.astype(NP_BF16).reshape(2, 128, n).transpose(1, 0, 2))


def kernel(x, x_mask, gamma, beta, Wp, bp, Wq, bq, Wk, bk, Wv, bv, Wo, bo):
    x = np.asarray(x, np.float32)
    m = np.asarray(x_mask, np.float32)
    gamma, beta, Wp, bp, Wq, bq, Wk, bk, Wv, bv, Wo, bo = (
        np.asarray(a, np.float32) for a in
        (gamma, beta, Wp, bp, Wq, bq, Wk, bk, Wv, bv, Wo, bo))

    assert not np.any(bk) and not np.any(bq) and not np.any(bv), \
        "nonzero conv biases not supported"
    assert not np.any(bp + Wp @ beta), "nonzero bp/beta not supported"

    Wp_g = Wp * gamma[None, :]
    ws = Wp_g.sum(axis=1)
    Wc = Wp_g - ws[:, None] / C
    WkC, WqC = Wk @ Wc, Wq @ Wc
    G = WkC.T @ WqC                  # scores = x'^T G x' (bilinear fold)
    wcat = np.stack([_chunk_pf((Wv @ Wc).T), _chunk_pf(Wo.T)],
                    axis=1)  # [128, 2, 2, 256]
    wcat = np.ascontiguousarray(wcat)
    const_vec = Wo @ bv + bo

    rstd = 1.0 / np.sqrt(x.var(axis=1) + EPS)      # [B, T]

    kept = [np.where(m[b, 0] > 0.5)[0] for b in range(B)]
    kcount = [len(k) for k in kept]
    k_pad = 256 * ((max(kcount) + 255) // 256)
    halves = [(kc + 1) // 2 for kc in kcount]
    q_pad = 128 * ((max(max(halves), max(kc - h for kc, h in
                                         zip(kcount, halves))) + 127) // 128)
    npair = k_pad // 256

    in_maps = []
    meta = []
    for core in range(N_CORES):
        b, half = divmod(core, 2)
        idx = kept[b]
        kc = kcount[b]
        qoff = 0 if half == 0 else halves[b]
        qn = halves[b] if half == 0 else kc - halves[b]
        ridx = np.roll(idx, -qoff)
        xp = np.zeros((C, k_pad), np.float32)
        xp[:, :kc] = x[b][:, ridx] * rstd[b][ridx][None, :]
        w1 = np.zeros((128, npair, 2, 16), NP_F8)
        rows = np.arange(128)
        for p in range(npair):
            for i in range(2):
                w1[:, p, i, :] = ((128 * (2 * p + i) + rows) < kc
                                  ).astype(NP_F8)[:, None]
        x8 = np.ascontiguousarray(
            xp.astype(NP_F8).reshape(2, 128, k_pad).transpose(1, 0, 2))
        up = np.zeros((C, q_pad), np.float32)
        nq = min(q_pad, kc)
        up[:, :nq] = G @ xp[:, :nq]          # u = G x' (fp32 host)
        u8 = np.ascontiguousarray(
            up.astype(NP_F8).reshape(2, 128, q_pad).transpose(1, 0, 2))
        in_maps.append({
            "x2": _chunk_pf(xp),
            "x8": x8,
            "u8": u8,
            "wcat": wcat,
            "w1": np.ascontiguousarray(w1),
        })
        meta.append((b, ridx, qn))

    nc = _get_nc(k_pad, q_pad)
    res = run_bass_kernel_spmd(nc, in_maps, list(range(N_CORES)))

    out = np.zeros((B, C, T), np.float32)
    for core in range(N_CORES):
        b, ridx, qn = meta[core]
        dev = np.asarray(res.results[core]["out"], np.float32)  # [q_pad, C]
        out[b][:, ridx[:qn]] = dev[:qn].T
    out += (x + const_vec[None, :, None])
    out *= m
    return out


# revision 6
# speedup vs baseline: 1.0020x; 1.0020x over previous
"""Trainium2 Bass kernel for nn_AttnBlock (B=4, C=256, T=4096) on 8 NeuronCores.

v2: fp8 DoubleRow attention + mask compaction.

Sharding: core = (batch b = core//2, query-half = core%2). The host compacts
each batch's time axis to its kept (mask=1) positions only — masked positions
are dead in the reference output (final *m) and as keys (softmax weight 0) —
then rolls the compacted axis so this core's query half sits at columns 0..Q.
Keys = all kept positions (padded to a multiple of 256), queries = this
core's half (padded to a multiple of 128).

Math foldings (exact):
  - gamma/beta/mean-subtraction fold into the conv weights (centered Wc).
  - The LayerNorm scale rstd commutes through every conv; the host folds it
    into the input once: x' = x * rstd. k/q/v then come straight from convs.
  - All biases are zero (asserted); Wo@bv+bo added on host.
  - softmax shift: e = exp(s/16 - SHIFT) fits fp8e4m3; shift cancels.
  - pad keys: x' columns are 0 so v-pad = 0 (kills AV) and the denominator
    ones-pattern has 0 rows at pads (kills the denom) — no -1e8 bias needed.

Dataflow per core (K_pad keys = NP pairs of 128-chunks, Q_pad queries in
tiles of <=512):
  convs (bf16, psum fp32): k8/q8 [c'(2x128), t] fp8; vt8 [s, chunk, c'] fp8
  per query tile jt, per key pair p:
    scores  = DoubleRow fp8: k8 pair -> sc [128, 2, qw] psum (2 banks)
    e       = one ACT exp per pair [128, 2, qw] -> fp8 (scalar bias/scale)
    AV m0   = DoubleRow fp8 into hpre bank;  m1 replayed after the loop
              (e tiles persist) to stay within 8 psum banks
    denom   = DoubleRow fp8 with ones-pattern weights -> dnm [16, qw] bank
  epilogue: hpre -> bf16, out^T = hpre^T @ Wo per 128-query chunk with
  per-partition 1/denom scale, bf16 DMA out.

ACT (exp) is the bottleneck engine (~59us); PE ~48us; DVE does all psum
drains (~39us); Pool cannot access PSUM.
"""
import sys

if "/opt/trn_rl_repo" not in sys.path:
    sys.path.insert(0, "/opt/trn_rl_repo")

import numpy as np
import ml_dtypes

import concourse.tile as tile
from concourse import bacc, mybir
from concourse.bass_utils import run_bass_kernel_spmd

B, C, T = 4, 256, 4096
N_CORES = 8
EPS = 1e-5
SCALE = float(C) ** -0.5
SHIFT = 4.0
BF16 = mybir.dt.bfloat16
F8 = mybir.dt.float8e4
F32 = mybir.dt.float32
NP_BF16 = ml_dtypes.bfloat16
NP_F8 = (ml_dtypes.float8_e4m3fn if hasattr(ml_dtypes, "float8_e4m3fn")
         else ml_dtypes.float8_e4m3)
DR = mybir.MatmulPerfMode.DoubleRow
AF = mybir.ActivationFunctionType


def _q_tiles(q_pad):
    """Query tile widths (each <=512, multiple of 128)."""
    tiles = [512] * (q_pad // 512)
    if q_pad % 512:
        tiles.append(q_pad % 512)
    return tiles


def build_kernel(k_pad, q_pad):
    npair = k_pad // 256
    nc = bacc.Bacc("TRN2", target_bir_lowering=False, debug=False,
                   num_devices=N_CORES)
    d_x2 = nc.dram_tensor("x2", [128, 2, k_pad], BF16, kind="ExternalInput").ap()
    d_x8 = nc.dram_tensor("x8", [128, 2, k_pad], F8, kind="ExternalInput").ap()
    d_u8 = nc.dram_tensor("u8", [128, 2, q_pad], F8, kind="ExternalInput").ap()
    d_w = nc.dram_tensor("wcat", [128, 2, 2, 256], BF16,
                         kind="ExternalInput").ap()
    d_w1 = nc.dram_tensor("w1", [128, npair, 2, 16], F8,
                          kind="ExternalInput").ap()
    d_out = nc.dram_tensor("out", [q_pad, C], BF16, kind="ExternalOutput").ap()
    with tile.TileContext(nc) as tc:
        _body(tc, d_x2, d_x8, d_u8, d_w, d_w1, d_out, k_pad, q_pad)
    nc.compile()
    return nc


def _body(tc, d_x2, d_x8, d_u8, d_w, d_w1, d_out, k_pad, q_pad):
    nc = tc.nc
    from contextlib import ExitStack

    npair = k_pad // 256
    qtiles = _q_tiles(q_pad)
    njt = len(qtiles)

    with ExitStack() as ctx:
        consts = ctx.enter_context(tc.tile_pool(name="consts", bufs=1))
        big = ctx.enter_context(tc.tile_pool(name="big", bufs=1))

        # ---- input DMAs (few large transfers) ----
        wcat = consts.tile([128, 2, 2, 256], BF16, tag="wcat")
        nc.gpsimd.dma_start(wcat[:], d_w[:])
        w1 = consts.tile([128, npair, 2, 16], F8, tag="w1")
        nc.gpsimd.dma_start(w1[:], d_w1[:])
        x8 = consts.tile([128, 2, k_pad], F8, tag="x8")
        u8 = consts.tile([128, 2, q_pad], F8, tag="u8")
        x2 = consts.tile([128, 2, k_pad], BF16, tag="x2")
        # u8 (host-computed G@x') + first x8 piece gate the first score
        # pair; x2 only feeds the v-convs
        nc.sync.dma_start(u8[:, :, 0:qtiles[0]], d_u8[:, :, 0:qtiles[0]])
        nc.sync.dma_start(x8[:, :, 0:512], d_x8[:, :, 0:512])
        nc.sync.dma_start(x2[:, :, 0:1024], d_x2[:, :, 0:1024])
        nc.sync.dma_start(x8[:, :, 512:k_pad], d_x8[:, :, 512:k_pad])
        if q_pad > qtiles[0]:
            nc.sync.dma_start(u8[:, :, qtiles[0]:q_pad],
                              d_u8[:, :, qtiles[0]:q_pad])
        bnd = [1024, 2048, 3072, k_pad]
        for lo, hi in zip(bnd[:-1], bnd[1:]):
            nc.sync.dma_start(x2[:, :, lo:hi], d_x2[:, :, lo:hi])

        wv, wo = (wcat[:, i] for i in range(2))
        biascol = consts.tile([128, 1], F32, tag="biascol")
        nc.vector.memset(biascol[:], -SHIFT)
        ones11 = consts.tile([1, 1], F32, tag="ones11")
        nc.vector.memset(ones11[:], 1.0)

        # persistent activations
        vt8 = big.tile([128, k_pad // 128, 256], F8, tag="vt8")

        # PSUM: sc 2x2 + hpre(m0) 1 + h1 1 + dnm 1 + bankp 1 = 8 banks
        scp = ctx.enter_context(tc.tile_pool(name="scp", bufs=2, space="PSUM"))
        hpp = ctx.enter_context(tc.tile_pool(name="hpp", bufs=1, space="PSUM"))
        hp1 = ctx.enter_context(tc.tile_pool(name="hp1", bufs=1, space="PSUM"))
        dnp = ctx.enter_context(tc.tile_pool(name="dnp", bufs=1, space="PSUM"))
        bankp = ctx.enter_context(tc.tile_pool(name="bankp", bufs=1,
                                               space="PSUM"))
        # SBUF pools
        e_pool = ctx.enter_context(tc.tile_pool(name="e_pool",
                                                bufs=npair + 4))
        hsb = ctx.enter_context(tc.tile_pool(name="hsb", bufs=2))
        s3t = ctx.enter_context(tc.tile_pool(name="s3t", bufs=3))
        s3o = ctx.enter_context(tc.tile_pool(name="s3o", bufs=2))

        # conv psum scratch alternates between bankp and hp1 (hp1 only
        # becomes the m1-accumulator after all convs are done)
        conv_pools = [bankp, hp1]
        conv_pi = [0]
        drain_alt = [0]

        def conv_drain(dst_ap, src_ap):
            nc.vector.tensor_copy(dst_ap, src_ap)

        def conv_psum():
            pool = conv_pools[conv_pi[0] % 2]
            conv_pi[0] += 1
            return pool.tile([128, 512], F32, tag="bank" if pool is bankp
                             else "h1", name="cp")

        # ---------------- conv units ------------------------------------
        def conv_kq(dst, w2, lo, hi):
            # dst[c'-half m, lo:hi] for m in 0,1; contraction over 2x128 c_in
            for mm in range(2):
                pp = conv_psum()
                p_ap = pp[:, 0:hi - lo]
                ms = slice(128 * mm, 128 * (mm + 1))
                nc.tensor.matmul(p_ap, w2[:, 0, ms], x2[:, 0, lo:hi],
                                 start=True, stop=False, skip_group_check=True)
                nc.tensor.matmul(p_ap, w2[:, 1, ms], x2[:, 1, lo:hi],
                                 start=False, stop=True, skip_group_check=True)
                conv_drain(dst[:, mm, lo:hi], p_ap)

        def conv_v(p):
            # v^T chunks 2p, 2p+1: [s, c] layout, one drain per pair
            vp = conv_psum()
            vv = vp.rearrange("p (two c) -> p two c", two=2)
            for cj in range(2):
                j = 2 * p + cj
                sl = slice(128 * j, 128 * (j + 1))
                nc.tensor.matmul(vv[:, cj], x2[:, 0, sl], wv[:, 0],
                                 start=True, stop=False, skip_group_check=True)
                nc.tensor.matmul(vv[:, cj], x2[:, 1, sl], wv[:, 1],
                                 start=False, stop=True, skip_group_check=True)
            conv_drain(vt8[:, 2 * p:2 * p + 2, :], vv[:])

        # conv unit list: interleaved into jt0's pair loop
        conv_units = []
        kbnd = list(range(0, k_pad, 512)) + [k_pad]
        kspans = list(zip(kbnd[:-1], kbnd[1:]))
        qbnd = list(range(0, q_pad, 512)) + [q_pad]
        qspans = list(zip(qbnd[:-1], qbnd[1:]))
        for p in range(npair):
            conv_units.append(lambda p=p: conv_v(p))

        # ---------------- attention building blocks ---------------------
        state = {}

        def open_jt(jt):
            state[jt] = {"e": {},
                         "h0": hpp.tile([128, 512], F32, tag="h", name="h0"),
                         "dnm": dnp.tile([16, 512], F32, tag="dnm",
                                         name="dnm"),
                         "hsb": hsb.tile([128, 2, 512], BF16, tag="hsb",
                                         name="hsb")}

        def sc_pair(jt, p):
            qlo = sum(qtiles[:jt])
            qw = qtiles[jt]
            sc = scp.tile([128, 2, 512], F32, tag="sc", name="sc")
            for i in range(2):
                ch = 2 * p + i
                nc.tensor.matmul(sc[:, i, 0:qw],
                                 x8[:, :, 128 * ch:128 * (ch + 1)],
                                 u8[:, :, qlo:qlo + qw],
                                 start=True, stop=True, perf_mode=DR,
                                 skip_group_check=True)
            e = e_pool.tile([128, 2, 512], F8, tag="e", name="e")
            nc.scalar.activation(e[:, :, 0:qw], sc[:, :, 0:qw], AF.Exp,
                                 bias=biascol[:], scale=SCALE)
            state[jt]["e"][p] = e

        def av_m0(jt, p):
            st = state[jt]
            qw = qtiles[jt]
            nc.tensor.matmul(st["h0"][:, 0:qw], vt8[:, 2 * p:2 * p + 2, 0:128],
                             st["e"][p][:, :, 0:qw], start=(p == 0),
                             stop=(p == npair - 1), perf_mode=DR,
                             skip_group_check=True)

        def dnm_p(jt, p):
            st = state[jt]
            qw = qtiles[jt]
            nc.tensor.matmul(st["dnm"][:, 0:qw], w1[:, p],
                             st["e"][p][:, :, 0:qw], start=(p == 0),
                             stop=(p == npair - 1), perf_mode=DR,
                             skip_group_check=True)

        def av_m1_open(jt):
            state[jt]["h1"] = hp1.tile([128, 512], F32, tag="h1", name="h1")

        def av_m1_p(jt, p, pop=True):
            st = state[jt]
            qw = qtiles[jt]
            e = st["e"].pop(p) if pop else st["e"][p]
            nc.tensor.matmul(st["h1"][:, 0:qw], vt8[:, 2 * p:2 * p + 2,
                                                    128:256],
                             e[:, :, 0:qw], start=(p == 0),
                             stop=(p == npair - 1), perf_mode=DR,
                             skip_group_check=True)

        def drain_h0(jt):
            st = state[jt]
            qw = qtiles[jt]
            nc.vector.tensor_copy(st["hsb"][:, 0, 0:qw], st["h0"][:, 0:qw])

        def drain_h1(jt):
            st = state[jt]
            qw = qtiles[jt]
            nc.vector.tensor_copy(st["hsb"][:, 1, 0:qw], st["h1"][:, 0:qw])

        def drain_drow(jt):
            st = state[jt]
            qw = qtiles[jt]
            drow = s3t.tile([1, 512], F32, tag="drow", name="drow")
            nc.vector.tensor_copy(drow[:, 0:qw], st["dnm"][0:1, 0:qw])
            st["drow"] = drow

        def rinv_jt(jt):
            st = state[jt]
            qw = qtiles[jt]
            nch = qw // 128
            dcol = bankp.tile([128, 512], F32, tag="bank", name="dcol")
            for c4 in range(nch):
                nc.tensor.matmul(dcol[:, c4:c4 + 1],
                                 st["drow"][0:1, 128 * c4:128 * (c4 + 1)],
                                 ones11[:], start=True, stop=True,
                                 skip_group_check=True)
            rinv = s3t.tile([128, 4], F32, tag="rinv", name="rinv")
            nc.vector.reciprocal(rinv[:, 0:nch], dcol[:, 0:nch])
            st["rinv"] = rinv

        def epilogue_open(jt):
            state[jt]["o_sb"] = s3o.tile([128, 4, 256], BF16, tag="o_sb",
                                         name="o_sb")

        def epilogue_chunk(jt, c4):
            st = state[jt]
            cs = slice(128 * c4, 128 * (c4 + 1))
            ot = bankp.tile([128, 512], F32, tag="bank", name="ot")
            nc.tensor.matmul(ot[:, 0:256], st["hsb"][:, 0, cs], wo[:, 0],
                             start=True, stop=False, skip_group_check=True)
            nc.tensor.matmul(ot[:, 0:256], st["hsb"][:, 1, cs], wo[:, 1],
                             start=False, stop=True, skip_group_check=True)
            nc.vector.tensor_scalar_mul(st["o_sb"][:, c4], ot[:, 0:256],
                                        st["rinv"][:, c4:c4 + 1])

        def epilogue_dma(jt):
            st = state.pop(jt)
            qlo = sum(qtiles[:jt])
            qw = qtiles[jt]
            nch = qw // 128
            dview = d_out[qlo:qlo + qw, :].rearrange("(c p) o -> p c o", p=128)
            nc.sync.dma_start(dview, st["o_sb"][:, 0:nch])

        # ---------------- emission --------------------------------------
        # jt0: conv units interleaved; av-m1 replayed at the jt0/jt1
        # boundary (hp1 doubles as conv scratch during jt0).
        # steady jts: av m0/m1/dnm all in-loop, lagged behind the scores so
        # prev-jt bank drains (emitted at jt start) land first.
        ci = 0

        def emit_convs(n):
            nonlocal ci
            for _ in range(n):
                if ci < len(conv_units):
                    conv_units[ci]()
                    ci += 1

        open_jt(0)
        sc_pair(0, 0)
        sc_pair(0, 1)
        emit_convs(1)               # v0
        for s in range(npair + 2):
            if s + 2 < npair:
                sc_pair(0, s + 2)
            emit_convs(2)
            if s >= 2:
                av_m0(0, s - 2)
                dnm_p(0, s - 2)
        emit_convs(len(conv_units))  # stragglers

        for jt in range(1, njt):
            open_jt(jt)
            prev = jt - 1
            sc_pair(jt, 0)
            sc_pair(jt, 1)
            if prev == 0:
                av_m1_open(0)       # replay jt0's m1 (e tiles persist)
                for p in range(npair):
                    av_m1_p(0, p)
            nch_prev = qtiles[prev] // 128
            for s in range(npair + 3):
                if s + 2 < npair:
                    sc_pair(jt, s + 2)
                if 2 <= s < npair + 2:
                    av_m0(jt, s - 2)
                    dnm_p(jt, s - 2)
                if s == 3:
                    av_m1_open(jt)
                if 3 <= s < npair + 3:
                    av_m1_p(jt, s - 3)
                if s == 0:
                    drain_h0(prev)
                elif s == 1:
                    drain_h1(prev)
                    drain_drow(prev)
                elif s == 2:
                    rinv_jt(prev)
                    epilogue_open(prev)
                elif 3 <= s < 3 + nch_prev:
                    epilogue_chunk(prev, s - 3)
                elif s == 3 + nch_prev:
                    epilogue_dma(prev)

        # ---- tail: last jt epilogue on the critical path ----
        last = njt - 1
        st = state[last]
        qw = qtiles[last]
        nch = qw // 128
        qlo = sum(qtiles[:last])
        drain_drow(last)            # gates the denom->rinv->scale chain
        nc.scalar.copy(st["hsb"][:, 1, 0:qw], st["h1"][:, 0:qw])   # ACT
        drain_h0(last)                                             # DVE
        rinv_jt(last)
        epilogue_open(last)
        dview = d_out[qlo:qlo + qw, :].rearrange("(c p) o -> p c o", p=128)
        for c4 in range(nch):
            cs = slice(128 * c4, 128 * (c4 + 1))
            pool = [bankp, hpp][c4 % 2]
            ot = pool.tile([128, 512], F32, tag="bank" if pool is bankp
                           else "h", name="ot")
            nc.tensor.matmul(ot[:, 0:256], st["hsb"][:, 0, cs], wo[:, 0],
                             start=True, stop=False, skip_group_check=True)
            nc.tensor.matmul(ot[:, 0:256], st["hsb"][:, 1, cs], wo[:, 1],
                             start=False, stop=True, skip_group_check=True)
            if c4 % 2 == 0:
                nc.vector.tensor_scalar_mul(st["o_sb"][:, c4], ot[:, 0:256],
                                            st["rinv"][:, c4:c4 + 1])
            else:
                nc.scalar.activation(st["o_sb"][:, c4], ot[:, 0:256],
                                     AF.Copy, bias=0.0,
                                     scale=st["rinv"][:, c4:c4 + 1])
            eng = [nc.sync, nc.gpsimd, nc.sync][c4 % 3]
            eng.dma_start(dview[:, c4:c4 + 1], st["o_sb"][:, c4:c4 + 1])
        state.pop(last)


_NC_CACHE = {}


def _get_nc(k_pad, q_pad):
    key = (k_pad, q_pad)
    if key not in _NC_CACHE:
        _NC_CACHE[key] = build_kernel(k_pad, q_pad)
    return _NC_CACHE[key]


def _chunk_pf(a):
    """[256, n] -> [128, 2, n] partition-first bf16."""
    n = a.shape[1]
    return np.ascontiguousarray(
        a.astype(NP_BF16).reshape(2, 128, n).transpose(1, 0, 2))


def kernel(x, x_mask, gamma, beta, Wp, bp, Wq, bq, Wk, bk, Wv, bv, Wo, bo):
    x = np.asarray(x, np.float32)
    m = np.asarray(x_mask, np.float32)
    gamma, beta, Wp, bp, Wq, bq, Wk, bk, Wv, bv, Wo, bo = (
        np.asarray(a, np.float32) for a in
        (gamma, beta, Wp, bp, Wq, bq, Wk, bk, Wv, bv, Wo, bo))

    assert not np.any(bk) and not np.any(bq) and not np.any(bv), \
        "nonzero conv biases not supported"
    assert not np.any(bp + Wp @ beta), "nonzero bp/beta not supported"

    Wp_g = Wp * gamma[None, :]
    ws = Wp_g.sum(axis=1)
    Wc = Wp_g - ws[:, None] / C
    WkC, WqC = Wk @ Wc, Wq @ Wc
    G = WkC.T @ WqC                  # scores = x'^T G x' (bilinear fold)
    wcat = np.stack([_chunk_pf((Wv @ Wc).T), _chunk_pf(Wo.T)],
                    axis=1)  # [128, 2, 2, 256]
    wcat = np.ascontiguousarray(wcat)
    const_vec = Wo @ bv + bo

    rstd = 1.0 / np.sqrt(x.var(axis=1) + EPS)      # [B, T]

    kept = [np.where(m[b, 0] > 0.5)[0] for b in range(B)]
    kcount = [len(k) for k in kept]
    k_pad = 256 * ((max(kcount) + 255) // 256)
    halves = [(kc + 1) // 2 for kc in kcount]
    q_pad = 128 * ((max(max(halves), max(kc - h for kc, h in
                                         zip(kcount, halves))) + 127) // 128)
    npair = k_pad // 256

    in_maps = []
    meta = []
    for core in range(N_CORES):
        b, half = divmod(core, 2)
        idx = kept[b]
        kc = kcount[b]
        qoff = 0 if half == 0 else halves[b]
        qn = halves[b] if half == 0 else kc - halves[b]
        ridx = np.roll(idx, -qoff)
        xp = np.zeros((C, k_pad), np.float32)
        xp[:, :kc] = x[b][:, ridx] * rstd[b][ridx][None, :]
        w1 = np.zeros((128, npair, 2, 16), NP_F8)
        rows = np.arange(128)
        for p in range(npair):
            for i in range(2):
                w1[:, p, i, :] = ((128 * (2 * p + i) + rows) < kc
                                  ).astype(NP_F8)[:, None]
        x8 = np.ascontiguousarray(
            xp.astype(NP_F8).reshape(2, 128, k_pad).transpose(1, 0, 2))
        up = np.zeros((C, q_pad), np.float32)
        nq = min(q_pad, kc)
        up[:, :nq] = G @ xp[:, :nq]          # u = G x' (fp32 host)
        u8 = np.ascontiguousarray(
            up.astype(NP_F8).reshape(2, 128, q_pad).transpose(1, 0, 2))
        in_maps.append({
            "x2": _chunk_pf(xp),
            "x8": x8,
            "u8": u8,
            "wcat": wcat,
            "w1": np.ascontiguousarray(w1),
        })
        meta.append((b, ridx, qn))

    nc = _get_nc(k_pad, q_pad)
    res = run_bass_kernel_spmd(nc, in_maps, list(range(N_CORES)))

    out = np.zeros((B, C, T), np.float32)
    for core in range(N_CORES):
        b, ridx, qn = meta[core]
        dev = np.asarray(res.results[core]["out"], np.float32)  # [q_pad, C]
        out[b][:, ridx[:qn]] = dev[:qn].T
    out += (x + const_vec[None, :, None])
    out *= m
    return out


# revision 7
# speedup vs baseline: 1.0089x; 1.0070x over previous
"""Trainium2 Bass kernel for nn_AttnBlock (B=4, C=256, T=4096) on 8 NeuronCores.

v2: fp8 DoubleRow attention + mask compaction.

Sharding: core = (batch b = core//2, query-half = core%2). The host compacts
each batch's time axis to its kept (mask=1) positions only — masked positions
are dead in the reference output (final *m) and as keys (softmax weight 0) —
then rolls the compacted axis so this core's query half sits at columns 0..Q.
Keys = all kept positions (padded to a multiple of 256), queries = this
core's half (padded to a multiple of 128).

Math foldings (exact):
  - gamma/beta/mean-subtraction fold into the conv weights (centered Wc).
  - The LayerNorm scale rstd commutes through every conv; the host folds it
    into the input once: x' = x * rstd. k/q/v then come straight from convs.
  - All biases are zero (asserted); Wo@bv+bo added on host.
  - softmax shift: e = exp(s/16 - SHIFT) fits fp8e4m3; shift cancels.
  - pad keys: x' columns are 0 so v-pad = 0 (kills AV) and the denominator
    ones-pattern has 0 rows at pads (kills the denom) — no -1e8 bias needed.

Dataflow per core (K_pad keys = NP pairs of 128-chunks, Q_pad queries in
tiles of <=512):
  convs (bf16, psum fp32): k8/q8 [c'(2x128), t] fp8; vt8 [s, chunk, c'] fp8
  per query tile jt, per key pair p:
    scores  = DoubleRow fp8: k8 pair -> sc [128, 2, qw] psum (2 banks)
    e       = one ACT exp per pair [128, 2, qw] -> fp8 (scalar bias/scale)
    AV m0   = DoubleRow fp8 into hpre bank;  m1 replayed after the loop
              (e tiles persist) to stay within 8 psum banks
    denom   = DoubleRow fp8 with ones-pattern weights -> dnm [16, qw] bank
  epilogue: hpre -> bf16, out^T = hpre^T @ Wo per 128-query chunk with
  per-partition 1/denom scale, bf16 DMA out.

ACT (exp) is the bottleneck engine (~59us); PE ~48us; DVE does all psum
drains (~39us); Pool cannot access PSUM.
"""
import sys

if "/opt/trn_rl_repo" not in sys.path:
    sys.path.insert(0, "/opt/trn_rl_repo")

import numpy as np
import ml_dtypes

import concourse.tile as tile
from concourse import bacc, mybir
from concourse.bass_utils import run_bass_kernel_spmd

B, C, T = 4, 256, 4096
N_CORES = 8
EPS = 1e-5
SCALE = float(C) ** -0.5
SHIFT = 4.0
BF16 = mybir.dt.bfloat16
F8 = mybir.dt.float8e4
F32 = mybir.dt.float32
NP_BF16 = ml_dtypes.bfloat16
NP_F8 = (ml_dtypes.float8_e4m3fn if hasattr(ml_dtypes, "float8_e4m3fn")
         else ml_dtypes.float8_e4m3)
DR = mybir.MatmulPerfMode.DoubleRow
AF = mybir.ActivationFunctionType


def _q_tiles(q_pad):
    """Query tile widths (each <=512, multiple of 128)."""
    tiles = [512] * (q_pad // 512)
    if q_pad % 512:
        tiles.append(q_pad % 512)
    return tiles


def build_kernel(k_pad, q_pad):
    npair = k_pad // 256
    nc = bacc.Bacc("TRN2", target_bir_lowering=False, debug=False,
                   num_devices=N_CORES)
    d_x2 = nc.dram_tensor("x2", [128, 2, k_pad], BF16, kind="ExternalInput").ap()
    d_x8 = nc.dram_tensor("x8", [128, 2, k_pad], F8, kind="ExternalInput").ap()
    d_u8 = nc.dram_tensor("u8", [128, 2, q_pad], F8, kind="ExternalInput").ap()
    d_w = nc.dram_tensor("wcat", [128, 2, 2, 256], BF16,
                         kind="ExternalInput").ap()
    d_w1 = nc.dram_tensor("w1", [128, npair, 2, 16], F8,
                          kind="ExternalInput").ap()
    d_out = nc.dram_tensor("out", [q_pad, C], BF16, kind="ExternalOutput").ap()
    with tile.TileContext(nc) as tc:
        _body(tc, d_x2, d_x8, d_u8, d_w, d_w1, d_out, k_pad, q_pad)
    nc.compile()
    return nc


def _body(tc, d_x2, d_x8, d_u8, d_w, d_w1, d_out, k_pad, q_pad):
    nc = tc.nc
    from contextlib import ExitStack

    npair = k_pad // 256
    qtiles = _q_tiles(q_pad)
    njt = len(qtiles)

    with ExitStack() as ctx:
        consts = ctx.enter_context(tc.tile_pool(name="consts", bufs=1))
        big = ctx.enter_context(tc.tile_pool(name="big", bufs=1))

        # ---- input DMAs (few large transfers) ----
        wcat = consts.tile([128, 2, 2, 256], BF16, tag="wcat")
        nc.gpsimd.dma_start(wcat[:], d_w[:])
        w1 = consts.tile([128, npair, 2, 16], F8, tag="w1")
        nc.gpsimd.dma_start(w1[:], d_w1[:])
        x8 = consts.tile([128, 2, k_pad], F8, tag="x8")
        u8 = consts.tile([128, 2, q_pad], F8, tag="u8")
        x2 = consts.tile([128, 2, k_pad], BF16, tag="x2")
        # u8 (host-computed G@x') + first x8 piece gate the first score
        # pair; x2 only feeds the v-convs
        nc.sync.dma_start(u8[:, :, 0:qtiles[0]], d_u8[:, :, 0:qtiles[0]])
        nc.sync.dma_start(x8[:, :, 0:512], d_x8[:, :, 0:512])
        if q_pad > qtiles[0]:
            nc.sync.dma_start(u8[:, :, qtiles[0]:q_pad],
                              d_u8[:, :, qtiles[0]:q_pad])
        nc.sync.dma_start(x8[:, :, 512:k_pad], d_x8[:, :, 512:k_pad])
        bnd = [0, 1024, 2048, 3072, k_pad]
        for lo, hi in zip(bnd[:-1], bnd[1:]):
            nc.sync.dma_start(x2[:, :, lo:hi], d_x2[:, :, lo:hi])

        wv, wo = (wcat[:, i] for i in range(2))
        biascol = consts.tile([128, 1], F32, tag="biascol")
        nc.vector.memset(biascol[:], -SHIFT)
        ones11 = consts.tile([1, 1], F32, tag="ones11")
        nc.vector.memset(ones11[:], 1.0)

        # persistent activations
        vt8 = big.tile([128, k_pad // 128, 256], F8, tag="vt8")

        # PSUM: sc 2x2 + hpre(m0) 1 + h1 1 + dnm 1 + bankp 1 = 8 banks
        scp = ctx.enter_context(tc.tile_pool(name="scp", bufs=2, space="PSUM"))
        hpp = ctx.enter_context(tc.tile_pool(name="hpp", bufs=1, space="PSUM"))
        hp1 = ctx.enter_context(tc.tile_pool(name="hp1", bufs=1, space="PSUM"))
        dnp = ctx.enter_context(tc.tile_pool(name="dnp", bufs=1, space="PSUM"))
        bankp = ctx.enter_context(tc.tile_pool(name="bankp", bufs=1,
                                               space="PSUM"))
        # SBUF pools
        e_pool = ctx.enter_context(tc.tile_pool(name="e_pool",
                                                bufs=npair + 4))
        hsb = ctx.enter_context(tc.tile_pool(name="hsb", bufs=2))
        s3t = ctx.enter_context(tc.tile_pool(name="s3t", bufs=3))
        s3o = ctx.enter_context(tc.tile_pool(name="s3o", bufs=2))

        # conv psum scratch alternates between bankp and hp1 (hp1 only
        # becomes the m1-accumulator after all convs are done)
        conv_pools = [bankp, hp1]
        conv_pi = [0]
        drain_alt = [0]

        def conv_drain(dst_ap, src_ap):
            nc.vector.tensor_copy(dst_ap, src_ap)

        def conv_psum():
            pool = conv_pools[conv_pi[0] % 2]
            conv_pi[0] += 1
            return pool.tile([128, 512], F32, tag="bank" if pool is bankp
                             else "h1", name="cp")

        # ---------------- conv units ------------------------------------
        def conv_kq(dst, w2, lo, hi):
            # dst[c'-half m, lo:hi] for m in 0,1; contraction over 2x128 c_in
            for mm in range(2):
                pp = conv_psum()
                p_ap = pp[:, 0:hi - lo]
                ms = slice(128 * mm, 128 * (mm + 1))
                nc.tensor.matmul(p_ap, w2[:, 0, ms], x2[:, 0, lo:hi],
                                 start=True, stop=False, skip_group_check=True)
                nc.tensor.matmul(p_ap, w2[:, 1, ms], x2[:, 1, lo:hi],
                                 start=False, stop=True, skip_group_check=True)
                conv_drain(dst[:, mm, lo:hi], p_ap)

        def conv_v(p):
            # v^T chunks 2p, 2p+1: [s, c] layout, one drain per pair
            vp = conv_psum()
            vv = vp.rearrange("p (two c) -> p two c", two=2)
            for cj in range(2):
                j = 2 * p + cj
                sl = slice(128 * j, 128 * (j + 1))
                nc.tensor.matmul(vv[:, cj], x2[:, 0, sl], wv[:, 0],
                                 start=True, stop=False, skip_group_check=True)
                nc.tensor.matmul(vv[:, cj], x2[:, 1, sl], wv[:, 1],
                                 start=False, stop=True, skip_group_check=True)
            conv_drain(vt8[:, 2 * p:2 * p + 2, :], vv[:])

        # conv unit list: interleaved into jt0's pair loop
        conv_units = []
        kbnd = list(range(0, k_pad, 512)) + [k_pad]
        kspans = list(zip(kbnd[:-1], kbnd[1:]))
        qbnd = list(range(0, q_pad, 512)) + [q_pad]
        qspans = list(zip(qbnd[:-1], qbnd[1:]))
        for p in range(npair):
            conv_units.append(lambda p=p: conv_v(p))

        # ---------------- attention building blocks ---------------------
        state = {}

        def open_jt(jt):
            state[jt] = {"e": {},
                         "h0": hpp.tile([128, 512], F32, tag="h", name="h0"),
                         "dnm": dnp.tile([16, 512], F32, tag="dnm",
                                         name="dnm"),
                         "hsb": hsb.tile([128, 2, 512], BF16, tag="hsb",
                                         name="hsb")}

        def sc_pair(jt, p):
            qlo = sum(qtiles[:jt])
            qw = qtiles[jt]
            sc = scp.tile([128, 2, 512], F32, tag="sc", name="sc")
            for i in range(2):
                ch = 2 * p + i
                nc.tensor.matmul(sc[:, i, 0:qw],
                                 x8[:, :, 128 * ch:128 * (ch + 1)],
                                 u8[:, :, qlo:qlo + qw],
                                 start=True, stop=True, perf_mode=DR,
                                 skip_group_check=True)
            e = e_pool.tile([128, 2, 512], F8, tag="e", name="e")
            nc.scalar.activation(e[:, :, 0:qw], sc[:, :, 0:qw], AF.Exp,
                                 bias=biascol[:], scale=SCALE)
            state[jt]["e"][p] = e

        def av_m0(jt, p):
            st = state[jt]
            qw = qtiles[jt]
            nc.tensor.matmul(st["h0"][:, 0:qw], vt8[:, 2 * p:2 * p + 2, 0:128],
                             st["e"][p][:, :, 0:qw], start=(p == 0),
                             stop=(p == npair - 1), perf_mode=DR,
                             skip_group_check=True)

        def dnm_p(jt, p):
            st = state[jt]
            qw = qtiles[jt]
            nc.tensor.matmul(st["dnm"][:, 0:qw], w1[:, p],
                             st["e"][p][:, :, 0:qw], start=(p == 0),
                             stop=(p == npair - 1), perf_mode=DR,
                             skip_group_check=True)

        def av_m1_open(jt):
            state[jt]["h1"] = hp1.tile([128, 512], F32, tag="h1", name="h1")

        def av_m1_p(jt, p, pop=True):
            st = state[jt]
            qw = qtiles[jt]
            e = st["e"].pop(p) if pop else st["e"][p]
            nc.tensor.matmul(st["h1"][:, 0:qw], vt8[:, 2 * p:2 * p + 2,
                                                    128:256],
                             e[:, :, 0:qw], start=(p == 0),
                             stop=(p == npair - 1), perf_mode=DR,
                             skip_group_check=True)

        def drain_h0(jt):
            st = state[jt]
            qw = qtiles[jt]
            nc.vector.tensor_copy(st["hsb"][:, 0, 0:qw], st["h0"][:, 0:qw])

        def drain_h1(jt):
            st = state[jt]
            qw = qtiles[jt]
            nc.vector.tensor_copy(st["hsb"][:, 1, 0:qw], st["h1"][:, 0:qw])

        def drain_drow(jt):
            st = state[jt]
            qw = qtiles[jt]
            drow = s3t.tile([1, 512], F32, tag="drow", name="drow")
            nc.vector.tensor_copy(drow[:, 0:qw], st["dnm"][0:1, 0:qw])
            st["drow"] = drow

        def rinv_jt(jt):
            st = state[jt]
            qw = qtiles[jt]
            nch = qw // 128
            dcol = bankp.tile([128, 512], F32, tag="bank", name="dcol")
            for c4 in range(nch):
                nc.tensor.matmul(dcol[:, c4:c4 + 1],
                                 st["drow"][0:1, 128 * c4:128 * (c4 + 1)],
                                 ones11[:], start=True, stop=True,
                                 skip_group_check=True)
            rinv = s3t.tile([128, 4], F32, tag="rinv", name="rinv")
            nc.vector.reciprocal(rinv[:, 0:nch], dcol[:, 0:nch])
            st["rinv"] = rinv

        def epilogue_open(jt):
            state[jt]["o_sb"] = s3o.tile([128, 4, 256], BF16, tag="o_sb",
                                         name="o_sb")

        def epilogue_chunk(jt, c4):
            st = state[jt]
            cs = slice(128 * c4, 128 * (c4 + 1))
            ot = bankp.tile([128, 512], F32, tag="bank", name="ot")
            nc.tensor.matmul(ot[:, 0:256], st["hsb"][:, 0, cs], wo[:, 0],
                             start=True, stop=False, skip_group_check=True)
            nc.tensor.matmul(ot[:, 0:256], st["hsb"][:, 1, cs], wo[:, 1],
                             start=False, stop=True, skip_group_check=True)
            nc.vector.tensor_scalar_mul(st["o_sb"][:, c4], ot[:, 0:256],
                                        st["rinv"][:, c4:c4 + 1])

        def epilogue_dma(jt):
            st = state.pop(jt)
            qlo = sum(qtiles[:jt])
            qw = qtiles[jt]
            nch = qw // 128
            dview = d_out[qlo:qlo + qw, :].rearrange("(c p) o -> p c o", p=128)
            nc.sync.dma_start(dview, st["o_sb"][:, 0:nch])

        # ---------------- emission --------------------------------------
        # jt0: conv units interleaved; av-m1 replayed at the jt0/jt1
        # boundary (hp1 doubles as conv scratch during jt0).
        # steady jts: av m0/m1/dnm all in-loop, lagged behind the scores so
        # prev-jt bank drains (emitted at jt start) land first.
        ci = 0

        def emit_convs(n):
            nonlocal ci
            for _ in range(n):
                if ci < len(conv_units):
                    conv_units[ci]()
                    ci += 1

        open_jt(0)
        sc_pair(0, 0)
        emit_convs(1)               # v0
        sc_pair(0, 1)
        for s in range(npair + 2):
            emit_convs(2)
            if s + 2 < npair:
                sc_pair(0, s + 2)
            if s >= 2:
                av_m0(0, s - 2)
                dnm_p(0, s - 2)
        emit_convs(len(conv_units))  # stragglers

        for jt in range(1, njt):
            open_jt(jt)
            prev = jt - 1
            sc_pair(jt, 0)
            sc_pair(jt, 1)
            if prev == 0:
                av_m1_open(0)       # replay jt0's m1 (e tiles persist)
                for p in range(npair):
                    av_m1_p(0, p)
            nch_prev = qtiles[prev] // 128
            for s in range(npair + 3):
                if s + 2 < npair:
                    sc_pair(jt, s + 2)
                if 2 <= s < npair + 2:
                    av_m0(jt, s - 2)
                    dnm_p(jt, s - 2)
                if s == 3:
                    av_m1_open(jt)
                if 3 <= s < npair + 3:
                    av_m1_p(jt, s - 3)
                if s == 0:
                    drain_h0(prev)
                elif s == 1:
                    drain_h1(prev)
                    drain_drow(prev)
                elif s == 2:
                    rinv_jt(prev)
                    epilogue_open(prev)
                elif 3 <= s < 3 + nch_prev:
                    epilogue_chunk(prev, s - 3)
                elif s == 3 + nch_prev:
                    epilogue_dma(prev)

        # ---- tail: last jt epilogue on the critical path ----
        last = njt - 1
        st = state[last]
        qw = qtiles[last]
        nch = qw // 128
        qlo = sum(qtiles[:last])
        drain_drow(last)            # gates the denom->rinv->scale chain
        nc.scalar.copy(st["hsb"][:, 1, 0:qw], st["h1"][:, 0:qw])   # ACT
        drain_h0(last)                                             # DVE
        rinv_jt(last)
        epilogue_open(last)
        dview = d_out[qlo:qlo + qw, :].rearrange("(c p) o -> p c o", p=128)
        for c4 in range(nch):
            cs = slice(128 * c4, 128 * (c4 + 1))
            pool = [bankp, hpp][c4 % 2]
            ot = pool.tile([128, 512], F32, tag="bank" if pool is bankp
                           else "h", name="ot")
            nc.tensor.matmul(ot[:, 0:256], st["hsb"][:, 0, cs], wo[:, 0],
                             start=True, stop=False, skip_group_check=True)
            nc.tensor.matmul(ot[:, 0:256], st["hsb"][:, 1, cs], wo[:, 1],
                             start=False, stop=True, skip_group_check=True)
            if c4 % 2 == 0:
                nc.vector.tensor_scalar_mul(st["o_sb"][:, c4], ot[:, 0:256],
                                            st["rinv"][:, c4:c4 + 1])
            else:
                nc.scalar.activation(st["o_sb"][:, c4], ot[:, 0:256],
                                     AF.Copy, bias=0.0,
                                     scale=st["rinv"][:, c4:c4 + 1])
            eng = [nc.sync, nc.gpsimd, nc.sync][c4 % 3]
            eng.dma_start(dview[:, c4:c4 + 1], st["o_sb"][:, c4:c4 + 1])
        state.pop(last)


_NC_CACHE = {}


def _get_nc(k_pad, q_pad):
    key = (k_pad, q_pad)
    if key not in _NC_CACHE:
        _NC_CACHE[key] = build_kernel(k_pad, q_pad)
    return _NC_CACHE[key]


def _chunk_pf(a):
    """[256, n] -> [128, 2, n] partition-first bf16."""
    n = a.shape[1]
    return np.ascontiguousarray(
        a.astype(NP_BF16).reshape(2, 128, n).transpose(1, 0, 2))


def kernel(x, x_mask, gamma, beta, Wp, bp, Wq, bq, Wk, bk, Wv, bv, Wo, bo):
    x = np.asarray(x, np.float32)
    m = np.asarray(x_mask, np.float32)
    gamma, beta, Wp, bp, Wq, bq, Wk, bk, Wv, bv, Wo, bo = (
        np.asarray(a, np.float32) for a in
        (gamma, beta, Wp, bp, Wq, bq, Wk, bk, Wv, bv, Wo, bo))

    assert not np.any(bk) and not np.any(bq) and not np.any(bv), \
        "nonzero conv biases not supported"
    assert not np.any(bp + Wp @ beta), "nonzero bp/beta not supported"

    Wp_g = Wp * gamma[None, :]
    ws = Wp_g.sum(axis=1)
    Wc = Wp_g - ws[:, None] / C
    WkC, WqC = Wk @ Wc, Wq @ Wc
    G = WkC.T @ WqC                  # scores = x'^T G x' (bilinear fold)
    wcat = np.stack([_chunk_pf((Wv @ Wc).T), _chunk_pf(Wo.T)],
                    axis=1)  # [128, 2, 2, 256]
    wcat = np.ascontiguousarray(wcat)
    const_vec = Wo @ bv + bo

    rstd = 1.0 / np.sqrt(x.var(axis=1) + EPS)      # [B, T]

    kept = [np.where(m[b, 0] > 0.5)[0] for b in range(B)]
    kcount = [len(k) for k in kept]
    k_pad = 256 * ((max(kcount) + 255) // 256)
    halves = [(kc + 1) // 2 for kc in kcount]
    q_pad = 128 * ((max(max(halves), max(kc - h for kc, h in
                                         zip(kcount, halves))) + 127) // 128)
    npair = k_pad // 256

    in_maps = []
    meta = []
    for core in range(N_CORES):
        b, half = divmod(core, 2)
        idx = kept[b]
        kc = kcount[b]
        qoff = 0 if half == 0 else halves[b]
        qn = halves[b] if half == 0 else kc - halves[b]
        ridx = np.roll(idx, -qoff)
        xp = np.zeros((C, k_pad), np.float32)
        xp[:, :kc] = x[b][:, ridx] * rstd[b][ridx][None, :]
        w1 = np.zeros((128, npair, 2, 16), NP_F8)
        rows = np.arange(128)
        for p in range(npair):
            for i in range(2):
                w1[:, p, i, :] = ((128 * (2 * p + i) + rows) < kc
                                  ).astype(NP_F8)[:, None]
        x8 = np.ascontiguousarray(
            xp.astype(NP_F8).reshape(2, 128, k_pad).transpose(1, 0, 2))
        up = np.zeros((C, q_pad), np.float32)
        nq = min(q_pad, kc)
        up[:, :nq] = G @ xp[:, :nq]          # u = G x' (fp32 host)
        u8 = np.ascontiguousarray(
            up.astype(NP_F8).reshape(2, 128, q_pad).transpose(1, 0, 2))
        in_maps.append({
            "x2": _chunk_pf(xp),
            "x8": x8,
            "u8": u8,
            "wcat": wcat,
            "w1": np.ascontiguousarray(w1),
        })
        meta.append((b, ridx, qn))

    nc = _get_nc(k_pad, q_pad)
    res = run_bass_kernel_spmd(nc, in_maps, list(range(N_CORES)))

    out = np.zeros((B, C, T), np.float32)
    for core in range(N_CORES):
        b, ridx, qn = meta[core]
        dev = np.asarray(res.results[core]["out"], np.float32)  # [q_pad, C]
        out[b][:, ridx[:qn]] = dev[:qn].T
    out += (x + const_vec[None, :, None])
    out *= m
    return out
